# revision 30
# baseline (speedup 1.0000x reference)
"""Multi-head self-attention Trainium2 kernel (8-core SPMD, full IO).

Problem: x:(2,2048,1024) f32; Wq/Wk/Wv/Wo:(1024,1024); bo:(1024,)
  out = softmax((xWq)(xWk)^T / 8) (xWv) reshaped @ Wo + bo

Sharding: data parallel on batch N=2 x tensor parallel on 16 heads in
4 groups of 4 heads.  Core c handles batch c//4, heads [4*(c%4), 4*(c%4)+4).
Each core computes a partial fc_out product (2048,1024); the host sums the
4 head-group partials per batch and adds the bias.

v2 design (vs the phase-sequential baseline):
  - q-chunk-outer pipeline: per 512-token q-chunk, attention -> denominator
    reciprocal -> normalize -> fc_out -> y DMA, all overlapped with the next
    q-chunk's attention.  No global post-attention stall.
  - projections interleaved INTO the first q-chunk's m-loop so the scalar
    engine (exp) starts ~15us in instead of ~65us, and the PE stays
    continuously busy (p-state ramp: PE reaches 2.4GHz only after ~3us of
    gap-free execution).
  - denominator: ones-column of V makes row 64 of the O^T psum the softmax
    denominator; per q-chunk it is reciprocal'd on DVE (reciprocal_approx_fast)
    and broadcast across partitions with a K=1 ones matmul on the PE --
    no DRAM bounce.
  - scores are computed TRANSPOSED (S^T[k,q]) so exp runs on ACT out of psum
    [128,1024] (one inst per head-pair per k-chunk) and AV contracts k on
    partitions.  No max subtraction (scores ~N(0,1) after /8 scaling).
"""

import os

import numpy as np

import concourse.bass as bass
import concourse.tile as tile
from concourse import bacc, mybir
from concourse import bass_utils

F32 = mybir.dt.float32
F32R = mybir.dt.float32r
BF16 = mybir.dt.bfloat16

EMBED = 1024
SEQ = 2048
NB = 2  # batch
HEADS = 16
D = 64  # head dim
NCORES = 8
GROUPS = 4  # head groups (tensor parallel)
HG = HEADS // GROUPS  # heads per core = 4
DG = HG * D  # dims per core = 256
KC = EMBED // 128  # 8 contraction chunks for projections
TCH = 512  # token chunk (projection granularity == q-chunk granularity)
NT = SEQ // TCH  # 4 chunks
QC = 512  # q tokens per attention block

_MM_DTYPE_NAME = "bfloat16"

# set by run_cores(); test.py reads exec_time_ns from here
LAST_RESULTS = None
_CACHED_NC = {}


def build_nc():
    nc = bacc.Bacc("TRN2", target_bir_lowering=False, debug=False,
                   num_devices=NCORES)

    xT = nc.dram_tensor("xT", (EMBED, SEQ), BF16, kind="ExternalInput").ap()
    wq = nc.dram_tensor("wq", (EMBED, DG), BF16, kind="ExternalInput").ap()
    wk = nc.dram_tensor("wk", (EMBED, DG), BF16, kind="ExternalInput").ap()
    wv = nc.dram_tensor("wv", (EMBED, DG), BF16, kind="ExternalInput").ap()
    wo = nc.dram_tensor("wo", (DG, EMBED), BF16, kind="ExternalInput").ap()
    y = nc.dram_tensor("y", (SEQ, EMBED), F32, kind="ExternalOutput").ap()

    with tile.TileContext(nc) as tc:
        with (
            tc.tile_pool(name="weights", bufs=1) as wpool,
            tc.tile_pool(name="qk", bufs=1) as qkpool,
            tc.tile_pool(name="vpool", bufs=1) as vpool,
            tc.tile_pool(name="otpool", bufs=1) as otpool,
            tc.tile_pool(name="xchunk", bufs=1) as xpool,
            tc.tile_pool(name="epool", bufs=4) as epool,
            tc.tile_pool(name="stage", bufs=2) as stpool,
            tc.tile_pool(name="rbp", bufs=4) as rbpool,
            tc.tile_pool(name="den", bufs=1) as denpool,
            tc.tile_pool(name="ystage", bufs=3) as ypool,
            tc.tile_pool(name="ps_sc", bufs=2, space="PSUM") as psB,  # scores
            tc.tile_pool(name="ps_po", bufs=1, space="PSUM") as psA,  # O^T acc
            tc.tile_pool(name="ps_mc", bufs=2, space="PSUM") as psC,  # proj/fc/rb
        ):
            # ---- persistent tiles ----
            wq_sb = wpool.tile([128, KC, DG], BF16, name="wq_sb", tag="wq")
            wk_sb = wpool.tile([128, KC, DG], BF16, name="wk_sb", tag="wk")
            wv_sb = wpool.tile([128, KC, DG], BF16, name="wv_sb", tag="wv")
            wo_sb = wpool.tile([128, DG // 128, EMBED], BF16, name="wo_sb",
                               tag="wo")
            QTs = [qkpool.tile([128, 2, TCH], BF16, name=f"qt{t}", tag=f"qt{t}")
                   for t in range(NT)]
            KTs = [qkpool.tile([128, 2, TCH], BF16, name=f"kt{t}", tag=f"kt{t}")
                   for t in range(NT)]
            Vs = [vpool.tile([128, 4, HG, D + 1], BF16, name=f"v{t}",
                             tag=f"v{t}") for t in range(NT)]
            OT2 = otpool.tile([128, 2, SEQ], BF16, name="ot2", tag="ot2")
            xcs = [xpool.tile([128, KC, TCH], BF16, name=f"xc{t}", tag=f"xc{t}")
                   for t in range(NT)]
            # ones row for the K=1 reciprocal-broadcast matmul
            ones_b = denpool.tile([1, D], BF16, name="ones_b", tag="ones")
            warm_in = denpool.tile([1, 8], F32, name="warm_in", tag="wi")
            warm_out = denpool.tile([1, 8], BF16, name="warm_out", tag="wo2")

            dm_l = denpool.tile([128, 128], BF16, name="dm_l", tag="dml")
            dm_r = denpool.tile([128, 512], BF16, name="dm_r", tag="dmr")

            # ---- input DMAs, split across the two HWDGE queues ----
            # x0 kc-pieces alternate between queues so the K0 projection
            # chain starts mid-transfer; weights interleave by first use.
            xTr = xT.rearrange("(c p) s -> p c s", p=128)
            nc.sync.dma_start(out=wk_sb,
                              in_=wk.rearrange("(c p) n -> p c n", p=128))
            nc.scalar.dma_start(out=wq_sb,
                                in_=wq.rearrange("(c p) n -> p c n", p=128))
            for kc in range(KC):
                q = nc.sync if kc % 2 == 0 else nc.scalar
                q.dma_start(out=xcs[0][:, kc:kc + 1, :],
                            in_=xTr[:, kc:kc + 1, 0:TCH])
            nc.scalar.dma_start(out=wv_sb,
                                in_=wv.rearrange("(c p) n -> p c n", p=128))
            for t in range(1, NT):
                q = nc.sync if t % 2 == 1 else nc.scalar
                q.dma_start(out=xcs[t],
                            in_=xTr[:, :, t * TCH:(t + 1) * TCH])
            nc.scalar.dma_start(out=wo_sb,
                                in_=wo.rearrange("(c p) n -> p c n", p=128))

            # constants: ones column 0 of V (denominator lands in po row 0,
            # base partition 0, so the reciprocal runs in place); ACT
            # exp-table warmup; zero tiles for the PE p-state warmup matmuls
            nc.vector.memset(dm_l, 0.0)
            nc.vector.memset(dm_r, 0.0)
            for t in range(NT):
                nc.vector.memset(Vs[t][:, :, :, 0:1], 1.0)
            nc.vector.memset(ones_b, 1.0)
            nc.vector.memset(warm_in, 0.0)
            nc.scalar.activation(out=warm_out, in_=warm_in,
                                 func=mybir.ActivationFunctionType.Exp,
                                 scale=1.0)

            # PE p-state warmup: the tensor engine ramps 0.65 -> 1.2 -> 2.4GHz
            # only over ~3us of gap-free execution.  Burn the DMA-prologue
            # time ramping on throwaway matmuls so projections run at speed;
            # the x0-paced K0/Q0 chains then keep the ramp alive (sub-us DMA
            # waits don't reset it), so 10 suffice to cover the DMA latency.
            for _ in range(10):
                psd = psC.tile([128, 512], F32, name="pd", tag="pc")
                nc.tensor.matmul(psd, dm_l, dm_r, start=True, stop=True)

            # ---- projection pieces (emitted interleaved with attention) ----
            def emit_qk(wsb, dst, t, mt):
                ps = psC.tile([128, 512], F32, name="pp", tag="pc")
                for kc in range(KC):
                    nc.tensor.matmul(
                        ps,
                        wsb[:, kc, mt * 128:(mt + 1) * 128],
                        xcs[t][:, kc, :],
                        start=(kc == 0),
                        stop=(kc == KC - 1),
                    )
                nc.vector.tensor_copy(out=dst[t][:, mt, :], in_=ps)

            def emit_v(t, ti):
                ps = psC.tile([128, 512], F32, name="pv", tag="pc")
                for kc in range(KC):
                    nc.tensor.matmul(
                        ps[:, 0:DG],
                        xcs[t][:, kc, ti * 128:(ti + 1) * 128],
                        wv_sb[:, kc, :],
                        start=(kc == 0),
                        stop=(kc == KC - 1),
                    )
                nc.vector.tensor_copy(
                    out=Vs[t][:, ti, :, 1:D + 1],
                    in_=ps[:, 0:DG].rearrange("p (h d) -> p h d", h=HG))

            rdens = {}  # (qcb, hm) -> [rden_b tiles j=0,1], set at drain time

            def emit_rb_norm(qcb, hm):
                # broadcast each head's 1/den row onto its 64 dim-partitions
                # (two K=1 ones-matmuls), then normalize O^T in place
                qs = slice(qcb * QC, (qcb + 1) * QC)
                rb = psC.tile([128, QC], F32, name="rb", tag="pc")
                for j in range(2):
                    nc.tensor.matmul(rb[j * D:(j + 1) * D, :], ones_b,
                                     rdens[(qcb, hm)][j],
                                     start=True, stop=True)
                nc.vector.tensor_mul(OT2[:, hm, qs], OT2[:, hm, qs], rb)

            def emit_fc(qcb, k):
                # one (token-block, out-column-block) piece of the partial
                # fc_out for q-chunk qcb
                tt, nch = divmod(k, EMBED // 512)
                trow = qcb * QC + tt * 128
                ps = psC.tile([128, 512], F32, name="fo", tag="pc")
                for hm in range(2):
                    nc.tensor.matmul(
                        ps,
                        OT2[:, hm, trow:trow + 128],
                        wo_sb[:, hm, nch * 512:(nch + 1) * 512],
                        start=(hm == 0),
                        stop=(hm == 1),
                    )
                ys = ypool.tile([128, 512], F32, name="ys", tag="ys")
                nc.vector.tensor_copy(out=ys, in_=ps)
                nc.sync.dma_start(
                    out=y[trow:trow + 128, nch * 512:(nch + 1) * 512],
                    in_=ys)

            def emit_piece(p):
                kind = p[0]
                if kind == "K":
                    emit_qk(wk_sb, KTs, p[1], p[2])
                elif kind == "Q":
                    emit_qk(wq_sb, QTs, p[1], p[2])
                elif kind == "V":
                    emit_v(p[1], p[2])
                elif kind == "RB":
                    emit_rb_norm(p[1], p[2])
                elif kind == "FC":
                    emit_fc(p[1], p[2])

            # minimal prologue: just enough for (qc0, hm0) scores + first AV.
            # K0/Q0 mt0 run as interleaved kc-chains so both consume the
            # streaming x0 pieces as they land instead of serializing.
            psk = psC.tile([128, 512], F32, name="ppk", tag="pc")
            psq = psC.tile([128, 512], F32, name="ppq", tag="pc")
            for kc in range(KC):
                nc.tensor.matmul(psk, wk_sb[:, kc, 0:128], xcs[0][:, kc, :],
                                 start=(kc == 0), stop=(kc == KC - 1))
                nc.tensor.matmul(psq, wq_sb[:, kc, 0:128], xcs[0][:, kc, :],
                                 start=(kc == 0), stop=(kc == KC - 1))
            nc.vector.tensor_copy(out=KTs[0][:, 0, :], in_=psk)
            nc.vector.tensor_copy(out=QTs[0][:, 0, :], in_=psq)
            for p in (("K", 0, 1), ("V", 0, 0)):
                emit_piece(p)

            # remaining projections just-in-time inside (qc0, hm) m-loops
            # (chunk t's K before scores m=4t, V_ti before AV m=4t+ti);
            # q-chunk qcb's epilogue (normalize-broadcast RB, fc_out FC) is
            # deferred into qcb+1's m-loops so the PE queue never head-of-line
            # blocks on the DVE/DMA epilogue chain.
            schedule = {
                (0, 0): {
                    0: [("V", 0, 1)],
                    1: [("V", 0, 2), ("K", 1, 0)],
                    2: [("V", 0, 3), ("K", 1, 1)],
                    3: [("V", 1, 0)],
                    4: [("V", 1, 1), ("K", 2, 0)],
                    5: [("V", 1, 2), ("K", 2, 1)],
                    6: [("V", 1, 3)],
                    7: [("V", 2, 0), ("K", 3, 0)],
                    8: [("V", 2, 1), ("K", 3, 1)],
                    9: [("V", 2, 2)], 10: [("V", 2, 3)],
                    11: [("V", 3, 0)], 12: [("V", 3, 1)],
                    13: [("V", 3, 2)], 14: [("V", 3, 3)],
                    15: [("Q", 0, 1)],
                },
                (0, 1): {
                    1: [("Q", 1, 0)], 3: [("Q", 1, 1)],
                },
                (1, 0): {
                    1: [("RB", 0, 0)],
                    3: [("RB", 0, 1), ("Q", 2, 0)],
                    4: [("FC", 0, 0)], 5: [("FC", 0, 1)],
                    6: [("Q", 2, 1)],
                    7: [("FC", 0, 2)], 8: [("FC", 0, 3)],
                },
                (1, 1): {
                    1: [("Q", 3, 0)], 2: [("FC", 0, 4)],
                    3: [("Q", 3, 1)], 4: [("FC", 0, 5)],
                    5: [("FC", 0, 6)], 6: [("FC", 0, 7)],
                },
                (2, 0): {
                    1: [("RB", 1, 0)], 3: [("RB", 1, 1)],
                    4: [("FC", 1, 0)], 5: [("FC", 1, 1)],
                    7: [("FC", 1, 2)], 8: [("FC", 1, 3)],
                },
                (2, 1): {
                    2: [("FC", 1, 4)], 3: [("FC", 1, 5)],
                    5: [("FC", 1, 6)], 6: [("FC", 1, 7)],
                },
                (3, 0): {
                    1: [("RB", 2, 0)], 3: [("RB", 2, 1)],
                    4: [("FC", 2, 0)], 5: [("FC", 2, 1)],
                    7: [("FC", 2, 2)], 8: [("FC", 2, 3)],
                },
                (3, 1): {
                    2: [("FC", 2, 4)], 3: [("FC", 2, 5)],
                    5: [("FC", 2, 6)], 6: [("FC", 2, 7)],
                    # qc3/hm0's normalize can run during this m-loop; only
                    # hm1's epilogue + FC(3) remain for the tail
                    8: [("RB", 3, 0)],
                },
            }

            # ---- attention + per-q-chunk epilogue ----
            for qcb in range(SEQ // QC):
                qs = slice(qcb * QC, (qcb + 1) * QC)
                for hm in range(2):
                    po = [psA.tile([D + 1, QC], F32, name=f"po{j}",
                                   tag=f"po{j}") for j in range(2)]
                    for m in range(SEQ // 128):
                        for p in schedule.get((qcb, hm), {}).get(m, []):
                            emit_piece(p)
                        ps = psB.tile([128, 2 * QC], F32, name="sc", tag="sc")
                        for j in range(2):
                            nc.tensor.matmul(
                                ps[:, j * QC:(j + 1) * QC],
                                KTs[m // 4][j * D:(j + 1) * D, hm,
                                            (m % 4) * 128:(m % 4 + 1) * 128],
                                QTs[qcb][j * D:(j + 1) * D, hm, :],
                                start=True,
                                stop=True,
                            )
                        e = epool.tile([128, 2 * QC], BF16, name="e", tag="e")
                        nc.scalar.activation(
                            out=e, in_=ps,
                            func=mybir.ActivationFunctionType.Exp,
                            scale=1.0 / np.sqrt(D),
                        )
                        for j in range(2):
                            nc.tensor.matmul(
                                po[j],
                                Vs[m // 4][:, m % 4, 2 * hm + j, :],
                                e[:, j * QC:(j + 1) * QC],
                                start=(m == 0),
                                stop=(m == SEQ // 128 - 1),
                            )
                    # drain O^T (row 0 = denominator) for this (q-chunk,
                    # pair); the st copies free the po psum banks, then the
                    # denominator is reciprocal'd in place and staged to bf16
                    # for the RB broadcast matmul
                    sts = []
                    for j in range(2):
                        st = stpool.tile([D + 1, QC], F32, name="st", tag="st")
                        nc.vector.tensor_copy(out=st, in_=po[j])
                        sts.append(st)
                    rpair = []
                    for j, st in enumerate(sts):
                        # casting DMA (f32 -> bf16, partition remap) via
                        # software DGE on gpsimd
                        nc.gpsimd.dma_start(
                            out=OT2[j * D:(j + 1) * D, hm, qs],
                            in_=st[1:D + 1, :])
                        nc.vector.reciprocal_approx_fast(
                            out=st[0:1, :], in_=st[0:1, :])
                        rden_b = rbpool.tile([1, QC], BF16, name="rden_b",
                                             tag="rden")
                        nc.vector.tensor_copy(out=rden_b, in_=st[0:1, :])
                        rpair.append(rden_b)
                    rdens[(qcb, hm)] = rpair

            # tail: last q-chunk's hm1 epilogue + its fc_out
            emit_rb_norm(NT - 1, 1)
            for k in range(8):
                emit_fc(NT - 1, k)

    nc.compile()
    return nc


def shard_inputs(x, Wv, Wk, Wq, Wo):
    """Build the 8 per-core input maps."""
    import ml_dtypes
    wire = ml_dtypes.bfloat16
    in_maps = []
    for c in range(NCORES):
        n, g = divmod(c, GROUPS)
        cols = slice(g * DG, (g + 1) * DG)
        in_maps.append({
            "xT": np.ascontiguousarray(
                np.asarray(x[n], np.float32).T).astype(wire),
            "wq": np.ascontiguousarray(
                np.asarray(Wq, np.float32)[:, cols]).astype(wire),
            "wk": np.ascontiguousarray(
                np.asarray(Wk, np.float32)[:, cols]).astype(wire),
            "wv": np.ascontiguousarray(
                np.asarray(Wv, np.float32)[:, cols]).astype(wire),
            "wo": np.ascontiguousarray(
                np.asarray(Wo, np.float32)[cols, :]).astype(wire),
        })
    return in_maps


def kernel(x, Wv, Wk, Wq, Wo, bo):
    global LAST_RESULTS
    x = np.asarray(x, np.float32)
    in_maps = shard_inputs(x, Wv, Wk, Wq, Wo)

    if "nc" not in _CACHED_NC:
        _CACHED_NC["nc"] = build_nc()
    nc = _CACHED_NC["nc"]

    trace = os.environ.get("MHA_TRACE", "0") == "1"
    res = bass_utils.run_bass_kernel_spmd(
        nc, in_maps, core_ids=list(range(NCORES)), trace=trace)
    LAST_RESULTS = res

    bo = np.asarray(bo, np.float32)
    out = np.empty((NB, SEQ, EMBED), np.float32)
    for n in range(NB):
        acc = res.results[n * GROUPS]["y"].astype(np.float32).copy()
        for g in range(1, GROUPS):
            acc += res.results[n * GROUPS + g]["y"]
        out[n] = acc + bo[None, :]
    return out


# revision 34
# speedup vs baseline: 1.0217x; 1.0217x over previous
"""Multi-head self-attention Trainium2 kernel (8-core SPMD, full IO).

Problem: x:(2,2048,1024) f32; Wq/Wk/Wv/Wo:(1024,1024); bo:(1024,)
  out = softmax((xWq)(xWk)^T / 8) (xWv) reshaped @ Wo + bo

Sharding: data parallel on batch N=2 x tensor parallel on 16 heads in
4 groups of 4 heads.  Core c handles batch c//4, heads [4*(c%4), 4*(c%4)+4).
Each core computes a partial fc_out product (2048,1024); the host sums the
4 head-group partials per batch and adds the bias.

v2 design (vs the phase-sequential baseline):
  - q-chunk-outer pipeline: per 512-token q-chunk, attention -> denominator
    reciprocal -> normalize -> fc_out -> y DMA, all overlapped with the next
    q-chunk's attention.  No global post-attention stall.
  - projections interleaved INTO the first q-chunk's m-loop so the scalar
    engine (exp) starts ~15us in instead of ~65us, and the PE stays
    continuously busy (p-state ramp: PE reaches 2.4GHz only after ~3us of
    gap-free execution).
  - denominator: ones-column of V makes row 64 of the O^T psum the softmax
    denominator; per q-chunk it is reciprocal'd on DVE (reciprocal_approx_fast)
    and broadcast across partitions with a K=1 ones matmul on the PE --
    no DRAM bounce.
  - scores are computed TRANSPOSED (S^T[k,q]) so exp runs on ACT out of psum
    [128,1024] (one inst per head-pair per k-chunk) and AV contracts k on
    partitions.  No max subtraction (scores ~N(0,1) after /8 scaling).
"""

import os

import numpy as np

import concourse.bass as bass
import concourse.tile as tile
from concourse import bacc, mybir
from concourse import bass_utils

F32 = mybir.dt.float32
F32R = mybir.dt.float32r
BF16 = mybir.dt.bfloat16

EMBED = 1024
SEQ = 2048
NB = 2  # batch
HEADS = 16
D = 64  # head dim
NCORES = 8
GROUPS = 4  # head groups (tensor parallel)
HG = HEADS // GROUPS  # heads per core = 4
DG = HG * D  # dims per core = 256
KC = EMBED // 128  # 8 contraction chunks for projections
TCH = 512  # token chunk (projection granularity == q-chunk granularity)
NT = SEQ // TCH  # 4 chunks
QC = 512  # q tokens per attention block

_MM_DTYPE_NAME = "bfloat16"

# set by run_cores(); test.py reads exec_time_ns from here
LAST_RESULTS = None
_CACHED_NC = {}


def build_nc():
    nc = bacc.Bacc("TRN2", target_bir_lowering=False, debug=False,
                   num_devices=NCORES)

    xT = nc.dram_tensor("xT", (EMBED, SEQ), BF16, kind="ExternalInput").ap()
    wq = nc.dram_tensor("wq", (EMBED, DG), BF16, kind="ExternalInput").ap()
    wk = nc.dram_tensor("wk", (EMBED, DG), BF16, kind="ExternalInput").ap()
    wv = nc.dram_tensor("wv", (EMBED, DG), BF16, kind="ExternalInput").ap()
    wo = nc.dram_tensor("wo", (DG, EMBED), BF16, kind="ExternalInput").ap()
    # bf16 partials: the host gather sums 4 head-group partials per batch in
    # f32; the ~0.2% bf16 quantization is well inside the error budget
    y = nc.dram_tensor("y", (SEQ, EMBED), BF16, kind="ExternalOutput").ap()

    with tile.TileContext(nc) as tc:
        with (
            tc.tile_pool(name="weights", bufs=1) as wpool,
            tc.tile_pool(name="qk", bufs=1) as qkpool,
            tc.tile_pool(name="vpool", bufs=1) as vpool,
            tc.tile_pool(name="otpool", bufs=1) as otpool,
            tc.tile_pool(name="xchunk", bufs=1) as xpool,
            tc.tile_pool(name="epool", bufs=4) as epool,
            tc.tile_pool(name="stage", bufs=2) as stpool,
            tc.tile_pool(name="rbp", bufs=4) as rbpool,
            tc.tile_pool(name="den", bufs=1) as denpool,
            tc.tile_pool(name="ystage", bufs=3) as ypool,
            tc.tile_pool(name="ps_sc", bufs=2, space="PSUM") as psB,  # scores
            tc.tile_pool(name="ps_po", bufs=1, space="PSUM") as psA,  # O^T acc
            tc.tile_pool(name="ps_mc", bufs=2, space="PSUM") as psC,  # proj/fc/rb
        ):
            # ---- persistent tiles ----
            wq_sb = wpool.tile([128, KC, DG], BF16, name="wq_sb", tag="wq")
            wk_sb = wpool.tile([128, KC, DG], BF16, name="wk_sb", tag="wk")
            wv_sb = wpool.tile([128, KC, DG], BF16, name="wv_sb", tag="wv")
            wo_sb = wpool.tile([128, DG // 128, EMBED], BF16, name="wo_sb",
                               tag="wo")
            QTs = [qkpool.tile([128, 2, TCH], BF16, name=f"qt{t}", tag=f"qt{t}")
                   for t in range(NT)]
            KTs = [qkpool.tile([128, 2, TCH], BF16, name=f"kt{t}", tag=f"kt{t}")
                   for t in range(NT)]
            Vs = [vpool.tile([128, 4, HG, D + 1], BF16, name=f"v{t}",
                             tag=f"v{t}") for t in range(NT)]
            OT2 = otpool.tile([128, 2, SEQ], BF16, name="ot2", tag="ot2")
            xcs = [xpool.tile([128, KC, TCH], BF16, name=f"xc{t}", tag=f"xc{t}")
                   for t in range(NT)]
            # ones row for the K=1 reciprocal-broadcast matmul
            ones_b = denpool.tile([1, D], BF16, name="ones_b", tag="ones")
            warm_in = denpool.tile([1, 8], F32, name="warm_in", tag="wi")
            warm_out = denpool.tile([1, 8], BF16, name="warm_out", tag="wo2")

            dm_l = denpool.tile([128, 128], BF16, name="dm_l", tag="dml")
            dm_r = denpool.tile([128, 512], BF16, name="dm_r", tag="dmr")

            # ---- input DMAs, split across the two HWDGE queues ----
            # x0 kc-pieces alternate between queues so the K0 projection
            # chain starts mid-transfer; weights interleave by first use.
            xTr = xT.rearrange("(c p) s -> p c s", p=128)
            nc.sync.dma_start(out=wk_sb,
                              in_=wk.rearrange("(c p) n -> p c n", p=128))
            nc.scalar.dma_start(out=wq_sb,
                                in_=wq.rearrange("(c p) n -> p c n", p=128))
            for kc in range(KC):
                q = nc.sync if kc % 2 == 0 else nc.scalar
                q.dma_start(out=xcs[0][:, kc:kc + 1, :],
                            in_=xTr[:, kc:kc + 1, 0:TCH])
            nc.scalar.dma_start(out=wv_sb,
                                in_=wv.rearrange("(c p) n -> p c n", p=128))
            for t in range(1, NT):
                q = nc.sync if t % 2 == 1 else nc.scalar
                q.dma_start(out=xcs[t],
                            in_=xTr[:, :, t * TCH:(t + 1) * TCH])
            nc.scalar.dma_start(out=wo_sb,
                                in_=wo.rearrange("(c p) n -> p c n", p=128))

            # constants: ones column 0 of V (denominator lands in po row 0,
            # base partition 0, so the reciprocal runs in place); ACT
            # exp-table warmup; zero tiles for the PE p-state warmup matmuls
            nc.vector.memset(dm_l, 0.0)
            nc.vector.memset(dm_r, 0.0)
            for t in range(NT):
                nc.vector.memset(Vs[t][:, :, :, 0:1], 1.0)
            nc.vector.memset(ones_b, 1.0)
            nc.vector.memset(warm_in, 0.0)
            nc.scalar.activation(out=warm_out, in_=warm_in,
                                 func=mybir.ActivationFunctionType.Exp,
                                 scale=1.0)

            # PE p-state warmup: the tensor engine ramps 0.65 -> 1.2 -> 2.4GHz
            # only over ~3us of gap-free execution.  Burn the DMA-prologue
            # time ramping on throwaway matmuls so projections run at speed.
            for _ in range(20):
                psd = psC.tile([128, 512], F32, name="pd", tag="pc")
                nc.tensor.matmul(psd, dm_l, dm_r, start=True, stop=True)

            # ---- projection pieces (emitted interleaved with attention) ----
            def emit_qk(wsb, dst, t, mt):
                ps = psC.tile([128, 512], F32, name="pp", tag="pc")
                for kc in range(KC):
                    nc.tensor.matmul(
                        ps,
                        wsb[:, kc, mt * 128:(mt + 1) * 128],
                        xcs[t][:, kc, :],
                        start=(kc == 0),
                        stop=(kc == KC - 1),
                    )
                nc.vector.tensor_copy(out=dst[t][:, mt, :], in_=ps)

            def emit_v(t, ti):
                ps = psC.tile([128, 512], F32, name="pv", tag="pc")
                for kc in range(KC):
                    nc.tensor.matmul(
                        ps[:, 0:DG],
                        xcs[t][:, kc, ti * 128:(ti + 1) * 128],
                        wv_sb[:, kc, :],
                        start=(kc == 0),
                        stop=(kc == KC - 1),
                    )
                nc.vector.tensor_copy(
                    out=Vs[t][:, ti, :, 1:D + 1],
                    in_=ps[:, 0:DG].rearrange("p (h d) -> p h d", h=HG))

            rdens = {}  # (qcb, hm) -> [rden_b tiles j=0,1], set at drain time

            def emit_rb_norm(qcb, hm):
                # broadcast each head's 1/den row onto its 64 dim-partitions
                # (two K=1 ones-matmuls), then normalize O^T in place
                qs = slice(qcb * QC, (qcb + 1) * QC)
                rb = psC.tile([128, QC], F32, name="rb", tag="pc")
                for j in range(2):
                    nc.tensor.matmul(rb[j * D:(j + 1) * D, :], ones_b,
                                     rdens[(qcb, hm)][j],
                                     start=True, stop=True)
                nc.vector.tensor_mul(OT2[:, hm, qs], OT2[:, hm, qs], rb)

            def emit_fc(qcb, k):
                # one (token-block, out-column-block) piece of the partial
                # fc_out for q-chunk qcb
                tt, nch = divmod(k, EMBED // 512)
                trow = qcb * QC + tt * 128
                ps = psC.tile([128, 512], F32, name="fo", tag="pc")
                for hm in range(2):
                    nc.tensor.matmul(
                        ps,
                        OT2[:, hm, trow:trow + 128],
                        wo_sb[:, hm, nch * 512:(nch + 1) * 512],
                        start=(hm == 0),
                        stop=(hm == 1),
                    )
                ys = ypool.tile([128, 512], BF16, name="ys", tag="ys")
                nc.vector.tensor_copy(out=ys, in_=ps)
                nc.sync.dma_start(
                    out=y[trow:trow + 128, nch * 512:(nch + 1) * 512],
                    in_=ys)

            def emit_piece(p):
                kind = p[0]
                if kind == "K":
                    emit_qk(wk_sb, KTs, p[1], p[2])
                elif kind == "Q":
                    emit_qk(wq_sb, QTs, p[1], p[2])
                elif kind == "V":
                    emit_v(p[1], p[2])
                elif kind == "RB":
                    emit_rb_norm(p[1], p[2])
                elif kind == "FC":
                    emit_fc(p[1], p[2])

            # minimal prologue: just enough for (qc0, hm0) scores + first AV.
            # K0/Q0 mt0 run as interleaved kc-chains so both consume the
            # streaming x0 pieces as they land instead of serializing.
            psk = psC.tile([128, 512], F32, name="ppk", tag="pc")
            psq = psC.tile([128, 512], F32, name="ppq", tag="pc")
            for kc in range(KC):
                nc.tensor.matmul(psk, wk_sb[:, kc, 0:128], xcs[0][:, kc, :],
                                 start=(kc == 0), stop=(kc == KC - 1))
                nc.tensor.matmul(psq, wq_sb[:, kc, 0:128], xcs[0][:, kc, :],
                                 start=(kc == 0), stop=(kc == KC - 1))
            nc.vector.tensor_copy(out=KTs[0][:, 0, :], in_=psk)
            nc.vector.tensor_copy(out=QTs[0][:, 0, :], in_=psq)
            for p in (("K", 0, 1), ("V", 0, 0)):
                emit_piece(p)

            # remaining projections just-in-time inside (qc0, hm) m-loops
            # (chunk t's K before scores m=4t, V_ti before AV m=4t+ti);
            # q-chunk qcb's epilogue (normalize-broadcast RB, fc_out FC) is
            # deferred into qcb+1's m-loops so the PE queue never head-of-line
            # blocks on the DVE/DMA epilogue chain.
            schedule = {
                (0, 0): {
                    0: [("V", 0, 1)],
                    1: [("V", 0, 2), ("K", 1, 0)],
                    2: [("V", 0, 3), ("K", 1, 1)],
                    3: [("V", 1, 0)],
                    4: [("V", 1, 1), ("K", 2, 0)],
                    5: [("V", 1, 2), ("K", 2, 1)],
                    6: [("V", 1, 3)],
                    7: [("V", 2, 0), ("K", 3, 0)],
                    8: [("V", 2, 1), ("K", 3, 1)],
                    9: [("V", 2, 2)], 10: [("V", 2, 3)],
                    11: [("V", 3, 0)], 12: [("V", 3, 1)],
                    13: [("V", 3, 2)], 14: [("V", 3, 3)],
                    15: [("Q", 0, 1)],
                },
                (0, 1): {
                    1: [("Q", 1, 0)], 3: [("Q", 1, 1)],
                },
                (1, 0): {
                    1: [("RB", 0, 0)],
                    3: [("RB", 0, 1), ("Q", 2, 0)],
                    4: [("FC", 0, 0)], 5: [("FC", 0, 1)],
                    6: [("Q", 2, 1)],
                    7: [("FC", 0, 2)], 8: [("FC", 0, 3)],
                },
                (1, 1): {
                    1: [("Q", 3, 0)], 2: [("FC", 0, 4)],
                    3: [("Q", 3, 1)], 4: [("FC", 0, 5)],
                    5: [("FC", 0, 6)], 6: [("FC", 0, 7)],
                },
                (2, 0): {
                    1: [("RB", 1, 0)], 3: [("RB", 1, 1)],
                    4: [("FC", 1, 0)], 5: [("FC", 1, 1)],
                    7: [("FC", 1, 2)], 8: [("FC", 1, 3)],
                },
                (2, 1): {
                    2: [("FC", 1, 4)], 3: [("FC", 1, 5)],
                    5: [("FC", 1, 6)], 6: [("FC", 1, 7)],
                },
                (3, 0): {
                    1: [("RB", 2, 0)], 3: [("RB", 2, 1)],
                    4: [("FC", 2, 0)], 5: [("FC", 2, 1)],
                    7: [("FC", 2, 2)], 8: [("FC", 2, 3)],
                },
                (3, 1): {
                    2: [("FC", 2, 4)], 3: [("FC", 2, 5)],
                    5: [("FC", 2, 6)], 6: [("FC", 2, 7)],
                    # qc3/hm0's normalize can run during this m-loop; only
                    # hm1's epilogue + FC(3) remain for the tail
                    8: [("RB", 3, 0)],
                },
            }

            # ---- attention + per-q-chunk epilogue ----
            for qcb in range(SEQ // QC):
                qs = slice(qcb * QC, (qcb + 1) * QC)
                for hm in range(2):
                    po = [psA.tile([D + 1, QC], F32, name=f"po{j}",
                                   tag=f"po{j}") for j in range(2)]

                    def emit_av(e, m):
                        for j in range(2):
                            nc.tensor.matmul(
                                po[j],
                                Vs[m // 4][:, m % 4, 2 * hm + j, :],
                                e[:, j * QC:(j + 1) * QC],
                                start=(m == 0),
                                stop=(m == SEQ // 128 - 1),
                            )

                    # scores(m) issue ahead of AV(m-1): every AV gets a full
                    # slot of slack for its e-tile, and across (qc,hm)
                    # boundaries the PE keeps feeding ACT scores while the
                    # previous pair's po-drain copies complete
                    pend = None
                    for m in range(SEQ // 128):
                        for p in schedule.get((qcb, hm), {}).get(m, []):
                            emit_piece(p)
                        ps = psB.tile([128, 2 * QC], F32, name="sc", tag="sc")
                        for j in range(2):
                            nc.tensor.matmul(
                                ps[:, j * QC:(j + 1) * QC],
                                KTs[m // 4][j * D:(j + 1) * D, hm,
                                            (m % 4) * 128:(m % 4 + 1) * 128],
                                QTs[qcb][j * D:(j + 1) * D, hm, :],
                                start=True,
                                stop=True,
                            )
                        e = epool.tile([128, 2 * QC], BF16, name="e", tag="e")
                        nc.scalar.activation(
                            out=e, in_=ps,
                            func=mybir.ActivationFunctionType.Exp,
                            scale=1.0 / np.sqrt(D),
                        )
                        if pend is not None:
                            emit_av(*pend)
                        pend = (e, m)
                    emit_av(*pend)
                    # drain O^T (row 0 = denominator) for this (q-chunk,
                    # pair); the st copies free the po psum banks, then the
                    # denominator is reciprocal'd in place and staged to bf16
                    # for the RB broadcast matmul
                    sts = []
                    for j in range(2):
                        st = stpool.tile([D + 1, QC], F32, name="st", tag="st")
                        nc.vector.tensor_copy(out=st, in_=po[j])
                        sts.append(st)
                    rpair = []
                    for j, st in enumerate(sts):
                        # casting DMA (f32 -> bf16, partition remap) via
                        # software DGE on gpsimd
                        nc.gpsimd.dma_start(
                            out=OT2[j * D:(j + 1) * D, hm, qs],
                            in_=st[1:D + 1, :])
                        nc.vector.reciprocal_approx_fast(
                            out=st[0:1, :], in_=st[0:1, :])
                        rden_b = rbpool.tile([1, QC], BF16, name="rden_b",
                                             tag="rden")
                        nc.vector.tensor_copy(out=rden_b, in_=st[0:1, :])
                        rpair.append(rden_b)
                    rdens[(qcb, hm)] = rpair

            # tail: last q-chunk's hm1 epilogue + its fc_out
            emit_rb_norm(NT - 1, 1)
            for k in range(8):
                emit_fc(NT - 1, k)

    nc.compile()
    return nc


def shard_inputs(x, Wv, Wk, Wq, Wo):
    """Build the 8 per-core input maps."""
    import ml_dtypes
    wire = ml_dtypes.bfloat16
    in_maps = []
    for c in range(NCORES):
        n, g = divmod(c, GROUPS)
        cols = slice(g * DG, (g + 1) * DG)
        in_maps.append({
            "xT": np.ascontiguousarray(
                np.asarray(x[n], np.float32).T).astype(wire),
            "wq": np.ascontiguousarray(
                np.asarray(Wq, np.float32)[:, cols]).astype(wire),
            "wk": np.ascontiguousarray(
                np.asarray(Wk, np.float32)[:, cols]).astype(wire),
            "wv": np.ascontiguousarray(
                np.asarray(Wv, np.float32)[:, cols]).astype(wire),
            "wo": np.ascontiguousarray(
                np.asarray(Wo, np.float32)[cols, :]).astype(wire),
        })
    return in_maps


def kernel(x, Wv, Wk, Wq, Wo, bo):
    global LAST_RESULTS
    x = np.asarray(x, np.float32)
    in_maps = shard_inputs(x, Wv, Wk, Wq, Wo)

    if "nc" not in _CACHED_NC:
        _CACHED_NC["nc"] = build_nc()
    nc = _CACHED_NC["nc"]

    trace = os.environ.get("MHA_TRACE", "0") == "1"
    res = bass_utils.run_bass_kernel_spmd(
        nc, in_maps, core_ids=list(range(NCORES)), trace=trace)
    LAST_RESULTS = res

    bo = np.asarray(bo, np.float32)
    out = np.empty((NB, SEQ, EMBED), np.float32)
    for n in range(NB):
        acc = res.results[n * GROUPS]["y"].astype(np.float32).copy()
        for g in range(1, GROUPS):
            acc += res.results[n * GROUPS + g]["y"]
        out[n] = acc + bo[None, :]
    return out
